# revision 21
# baseline (speedup 1.0000x reference)
"""AFNO2D (FFT2 -> block-diag complex MLP -> softshrink -> IFFT2) on 8 TRN2
NeuronCores.

Sharding: 32 independent (batch, channel-block) units; core i handles
(b = i // 2, channel half = i % 2) = 4 blocks of 32 channels.  No collectives.

Per-core Bass pipeline (all FFTs as TensorEngine matmuls against DFT
matrices, bf16 operands, fp32 PSUM accumulation).  Key structure: the final
real(IFFT2) only needs the Hermitian-symmetrized spectrum.  Since softshrink
is odd, the mirror column's conjugation folds into the MLP weights, and the
symmetrization U = S[kw] + conj(S[256-kw] @ kh-reversed) is a single add
after softshrink.

  pass A: FFT along W; Hermitian symmetry of the real-input FFT -> two
          128-row outputs: R = re kw 0..127, I = [im kw 1..127, re kw 128]
          (Nyquist folded into the main matmul) -> DRAM (kw, h, c)
  pass B: per kw 1..127: radix-2 FFT along H (even/odd h-halves, twiddle
          folded into the odd DFT matrix, butterfly on DVE doubles as the
          PSUM->SBUF move); MLP for the column and its mirror (mirror
          emitted at reversed-kh order so its input is conj(z), handled by
          sign-swapped weights).  L2 with o1 as lhsT lands (kh, c); bias is
          added by the DVE PSUM-drain (no bias matmul); softshrink
          clamp/sub on GpSimd; U = S0 + M1 on DVE.  IFFT along H batched
          over unit pairs -> CAST into h-partition staging -> SBUF->SBUF
          scatter DMA into resident (kw-partition) ctP/ctQ tiles.
  pass C: IFFT along W straight out of SBUF: out = cgA.P + cgB.Q with
          cgA = cos rows kw 0..127, cgB = [-sin rows kw 1..127; cos row
          128]; real out (h, w, c) -> DRAM.  No input DMA.
"""

import os
import sys

sys.path.insert(0, "/opt/trn_rl_repo")

import numpy as np
import ml_dtypes

import concourse.bacc as bacc
import concourse.mybir as mybir
from concourse import tile
from concourse.alu_op_type import AluOpType
from concourse.bass_utils import run_bass_kernel_spmd

FT = mybir.dt.float32
BF = mybir.dt.bfloat16
AF = mybir.ActivationFunctionType
H = 256
W = 256
C = 128
LAM = 0.01


def _build():
    nc = bacc.Bacc("TRN2", target_bir_lowering=False, debug=False, num_devices=8)

    xt = nc.declare_dram_parameter("xt", [W, H, C], BF, isOutput=False)
    fwr = nc.declare_dram_parameter("fwr", [W, 128], BF, isOutput=False)   # (w, kw)
    fwi2 = nc.declare_dram_parameter("fwi2", [W, 128], BF, isOutput=False)
    fh1 = nc.declare_dram_parameter("fh1", [H, 2 * H], BF, isOutput=False)  # [Fr|Fi]
    fh2 = nc.declare_dram_parameter("fh2", [H, 2 * H], BF, isOutput=False)  # [-Fi|Fr]
    gr = nc.declare_dram_parameter("gr", [H, H], BF, isOutput=False)
    gi = nc.declare_dram_parameter("gi", [H, H], BF, isOutput=False)
    gn = nc.declare_dram_parameter("gn", [H, H], BF, isOutput=False)       # -Gi
    cga = nc.declare_dram_parameter("cga", [128, W], BF, isOutput=False)
    cgb = nc.declare_dram_parameter("cgb", [128, W], BF, isOutput=False)
    # L1: mixed matrices producing (o1r, o1i) interleaved by 32-block on the
    # output partition dim: tile X covers blocks (2X, 2X+1) as
    # [r_b(32) | i_b(32) | r_b+1 | i_b+1].  a1 multiplies zr (shared by the
    # mirror column), a2 multiplies zi (a2m = mirror variant).
    wmats = {}
    for name in ["a1_0", "a1_1", "a2_0", "a2_1", "a2m_0", "a2m_1"]:
        wmats[name] = nc.declare_dram_parameter(name, [C, C], BF, isOutput=False)
    # L2: packed [interleaved o1 rows] -> [re_b(32) | im_b(32) | ...] cols
    w2d = {}
    for name in ["w2_0", "w2_1", "w2m_0", "w2m_1"]:
        w2d[name] = nc.declare_dram_parameter(name, [C, C], BF, isOutput=False)
    b1_d = {}
    for name in ["b1_0", "b1_1"]:
        b1_d[name] = nc.declare_dram_parameter(name, [C, 1], FT, isOutput=False)
    # b2 replicated across partitions: added by the DVE PSUM-drain
    b2r_d = {}
    for name in ["b2rn", "b2rs"]:
        b2r_d[name] = nc.declare_dram_parameter(name, [C, 4 * C], BF,
                                                isOutput=False)
    out = nc.declare_dram_parameter("out", [W, H, C], BF, isOutput=True)

    sp1r = nc.dram_tensor("sp1r", [128, H, C], BF)        # (kw 0..127, h, c)
    sp1i = nc.dram_tensor("sp1i", [128, H, C], BF)        # (im kw 1..127 | re 128)
    sp2 = nc.dram_tensor("sp2", [H, 129, 2, C], BF)       # (h, kwcol, {P,Q}, c)

    with tile.TileContext(nc) as tc:
        with tc.tile_pool(name="const", bufs=1) as cp:
            fw_sb = {"r": [], "i": []}
            for comp, src in (("r", fwr), ("i", fwi2)):
                for hh in range(2):
                    t = cp.tile([128, 128], BF, name=f"fw{comp}{hh}", tag=f"fw{comp}{hh}")
                    nc.sync.dma_start(t[:], src[hh * 128:(hh + 1) * 128, :])
                    fw_sb[comp].append(t)
            fh_sb = {1: [], 2: []}
            for idx, src in ((1, fh1), (2, fh2)):
                for hh in range(2):
                    t = cp.tile([128, 2 * H], BF, name=f"fh{idx}{hh}", tag=f"fh{idx}{hh}")
                    nc.gpsimd.dma_start(t[:], src[hh * 128:(hh + 1) * 128, :])
                    fh_sb[idx].append(t)
            g_sb = {}
            for nm, src in (("gr", gr), ("gi", gi), ("gn", gn)):
                g_sb[nm] = []
                for hh in range(2):
                    t = cp.tile([128, H], BF, name=f"{nm}{hh}", tag=f"{nm}{hh}")
                    nc.gpsimd.dma_start(t[:], src[hh * 128:(hh + 1) * 128, :])
                    g_sb[nm].append(t)
            cg_sb = {}
            for nm, src in (("cga", cga), ("cgb", cgb)):
                t = cp.tile([128, W], BF, name=nm, tag=nm)
                nc.gpsimd.dma_start(t[:], src[:])
                cg_sb[nm] = t
            w_sb = {}
            for nm, src in wmats.items():
                t = cp.tile([C, C], BF, name=nm, tag=nm)
                nc.gpsimd.dma_start(t[:], src[:])
                w_sb[nm] = t
            w2_sb = {}
            for nm, src in w2d.items():
                t = cp.tile([C, C], BF, name=nm, tag=nm)
                nc.gpsimd.dma_start(t[:], src[:])
                w2_sb[nm] = t
            b1_sb = {}
            for nm, src in b1_d.items():
                t = cp.tile([C, 1], FT, name=nm, tag=nm)
                nc.gpsimd.dma_start(t[:], src[:])
                b1_sb[nm] = t
            b2r_sb = {}
            for nm, src in b2r_d.items():
                t = cp.tile([C, 4 * C], BF, name=nm, tag=nm)
                nc.gpsimd.dma_start(t[:], src[:])
                b2r_sb[nm] = t
            # resident IFFT-H outputs, pass-C layout (kw-partition):
            #   ctP rows = kw 0..127 (P comp), ctQ rows = kw 1..128 (Q comp)
            ctP = cp.tile([128, H, C], BF, name="ctP", tag="ctP")
            ctQ = cp.tile([128, H, C], BF, name="ctQ", tag="ctQ")

            # ---------------- pass A: FFT along W ----------------
            with (
                tc.tile_pool(name="a_in", bufs=4) as ain,
                tc.tile_pool(name="a_ev", bufs=8) as aev,
                tc.tile_pool(name="a_ps", bufs=6, space="PSUM") as aps,
            ):
                for g in range(32):           # 8 h-rows per group
                    xr = []
                    for wh in range(2):
                        t = ain.tile([128, 8, C], BF, name=f"xr{wh}", tag=f"xr{wh}")
                        eng = nc.sync if wh == 0 else nc.scalar
                        eng.dma_start(
                            t[:], xt[wh * 128:(wh + 1) * 128, 8 * g:8 * g + 8, :]
                        )
                        xr.append(t)
                    evs = {}
                    for comp in ("r", "i"):
                        evs[comp] = aev.tile([128, 8, C], BF, name=f"aev{comp}",
                                             tag=f"aev{comp}")
                    k = 0
                    for comp in ("r", "i"):
                        for sub in range(2):
                            ps = aps.tile([128, 4, C], FT, name="aps", tag="aps")
                            for wh in range(2):
                                nc.tensor.matmul(
                                    ps[:],
                                    fw_sb[comp][wh][:],
                                    xr[wh][:, 4 * sub:4 * sub + 4, :],
                                    start=(wh == 0),
                                    stop=(wh == 1),
                                )
                            if k % 2 == 0:
                                nc.vector.tensor_copy(
                                    evs[comp][:, 4 * sub:4 * sub + 4, :], ps[:])
                            else:
                                nc.scalar.copy(
                                    evs[comp][:, 4 * sub:4 * sub + 4, :], ps[:])
                            k += 1
                    weng = nc.sync if g >= 28 else nc.gpsimd
                    for comp, dst in (("r", sp1r), ("i", sp1i)):
                        weng.dma_start(
                            dst[0:128, 8 * g:8 * g + 8, :], evs[comp][:])

            # ---------------- pass B ----------------
            with (
                tc.tile_pool(name="b_in", bufs=2) as bin_,
                tc.tile_pool(name="b_s2", bufs=2) as bs2,
                tc.tile_pool(name="b_o1", bufs=3) as bo1,
                tc.tile_pool(name="b_pb", bufs=3) as bpb,
                tc.tile_pool(name="b_cl", bufs=3) as bcl,
                tc.tile_pool(name="b_ss", bufs=3) as bss,
                tc.tile_pool(name="b_yt", bufs=2) as byt,
                tc.tile_pool(name="b_v", bufs=4) as bv,
                tc.tile_pool(name="b_ps2", bufs=2, space="PSUM") as ps2p,
                tc.tile_pool(name="b_p1", bufs=1, space="PSUM") as p1p,
                tc.tile_pool(name="b_p2", bufs=2, space="PSUM") as p2p,
                tc.tile_pool(name="b_psv", bufs=2, space="PSUM") as psvp,
            ):

                def do_fft_h(bl, jj):
                    """FFT along H for one kw column -> s2 (c, [re 256|im 256])."""
                    ps = ps2p.tile([128, 2 * H], FT, name="s2ps", tag="s2ps")
                    nc.tensor.matmul(ps[:], bl[("r", 0)][:, jj, :],
                                     fh_sb[1][0][:], start=True, stop=False)
                    nc.tensor.matmul(ps[:], bl[("i", 0)][:, jj, :],
                                     fh_sb[2][0][:], start=False, stop=False)
                    nc.tensor.matmul(ps[:], bl[("r", 1)][:, jj, :],
                                     fh_sb[1][1][:], start=False, stop=False)
                    nc.tensor.matmul(ps[:], bl[("i", 1)][:, jj, :],
                                     fh_sb[2][1][:], start=False, stop=True)
                    s2 = bs2.tile([128, 2, H], BF, name="s2", tag="s2")
                    nc.scalar.copy(s2[:], ps[:])
                    return s2

                def do_fft_h_real(bsp, sl):
                    """FFT along H of a real column (bsp[:, sl])."""
                    ps = ps2p.tile([128, 2 * H], FT, name="s2ps", tag="s2ps")
                    nc.tensor.matmul(ps[:], bsp[0][:, sl, :], fh_sb[1][0][:],
                                     start=True, stop=False)
                    nc.tensor.matmul(ps[:], bsp[1][:, sl, :], fh_sb[1][1][:],
                                     start=False, stop=True)
                    s2 = bs2.tile([128, 2, H], BF, name="s2", tag="s2")
                    nc.scalar.copy(s2[:], ps[:])
                    return s2

                def do_l1(s2):
                    """L1 for a column and its mirror.  p1a = (normal-01,
                    mirror-01) sharing bias b1_0 -> one batched relu; p1b =
                    (normal-23, mirror-23) with b1_1."""
                    zr, zi = s2[:, 0, :], s2[:, 1, :]
                    p1a = p1p.tile([128, 2, H], FT, name="p1a", tag="p1a")
                    nc.tensor.matmul(p1a[:, 0], w_sb["a1_0"][:], zr,
                                     start=True, stop=False)
                    nc.tensor.matmul(p1a[:, 1], w_sb["a1_0"][:], zr,
                                     start=False, stop=False)
                    nc.tensor.matmul(p1a[:, 0], w_sb["a2_0"][:], zi,
                                     start=False, stop=False)
                    nc.tensor.matmul(p1a[:, 1], w_sb["a2m_0"][:], zi,
                                     start=False, stop=True)
                    p1b = p1p.tile([128, 2, H], FT, name="p1b", tag="p1b")
                    nc.tensor.matmul(p1b[:, 0], w_sb["a1_1"][:], zr,
                                     start=True, stop=False)
                    nc.tensor.matmul(p1b[:, 1], w_sb["a1_1"][:], zr,
                                     start=False, stop=False)
                    nc.tensor.matmul(p1b[:, 0], w_sb["a2_1"][:], zi,
                                     start=False, stop=False)
                    nc.tensor.matmul(p1b[:, 1], w_sb["a2m_1"][:], zi,
                                     start=False, stop=True)
                    o1a = bo1.tile([128, 2, H], BF, name="o1a", tag="o1a")
                    nc.scalar.activation(o1a[:], p1a[:], AF.Relu,
                                         bias=b1_sb["b1_0"][:])
                    o1b = bo1.tile([128, 2, H], BF, name="o1b", tag="o1b")
                    nc.scalar.activation(o1b[:], p1b[:], AF.Relu,
                                         bias=b1_sb["b1_1"][:])
                    return (o1a, o1b)

                def do_l1_real(s2a, s2b):
                    """L1 for the two real-input columns (kw 0 and 128): no
                    mirror, but z = FFT_H(re) is still complex."""
                    o1 = []
                    for s2 in (s2a, s2b):
                        zr, zi = s2[:, 0, :], s2[:, 1, :]
                        p1a = p1p.tile([128, 2, H], FT, name="p1a", tag="p1a")
                        nc.tensor.matmul(p1a[:, 0], w_sb["a1_0"][:], zr,
                                         start=True, stop=False)
                        nc.tensor.matmul(p1a[:, 1], w_sb["a1_1"][:], zr,
                                         start=False, stop=False)
                        nc.tensor.matmul(p1a[:, 0], w_sb["a2_0"][:], zi,
                                         start=False, stop=False)
                        nc.tensor.matmul(p1a[:, 1], w_sb["a2_1"][:], zi,
                                         start=False, stop=True)
                        o1u = []
                        for x in range(2):
                            t = bo1.tile([128, H], BF, name=f"o1n{x}",
                                         tag=f"o1n{x}")
                            nc.scalar.activation(t[:], p1a[:, x], AF.Relu,
                                                 bias=b1_sb[f"b1_{x}"][:])
                            o1u.append(t)
                        o1.append(o1u)
                    return o1

                def do_l2_unit(o1, ytk, up):
                    """L2 + bias + softshrink + U for one (column, mirror)
                    unit.  p2 cols per slot: (b: re32 | im32) x 4 blocks.
                    Bias is applied by the DVE PSUM-drain (no bias matmul);
                    clamp/sub run on GpSimd over both kh-halves at once."""
                    o1a, o1b = o1
                    pb = bpb.tile([128, 2, 2, 4, 2, 32], BF, name="pb", tag="pb")
                    for kb in range(2):
                        sl = slice(kb * 128, (kb + 1) * 128)
                        p2 = p2p.tile([128, 2, 2 * C], FT, name="p2", tag="p2")
                        nc.tensor.matmul(p2[:, 0, 0:128], o1a[:, 0, sl],
                                         w2_sb["w2_0"][:], start=True, stop=True)
                        nc.tensor.matmul(p2[:, 0, 128:256], o1b[:, 0, sl],
                                         w2_sb["w2_1"][:], start=True, stop=True)
                        nc.tensor.matmul(p2[:, 1, 0:128], o1a[:, 1, sl],
                                         w2_sb["w2m_0"][:], start=True, stop=True)
                        nc.tensor.matmul(p2[:, 1, 128:256], o1b[:, 1, sl],
                                         w2_sb["w2m_1"][:], start=True, stop=True)
                        nc.vector.tensor_tensor(pb[:, kb], p2[:],
                                                b2r_sb["b2rn"][:],
                                                AluOpType.add)
                    cl = bcl.tile([128, 2, 2, 4, 2, 32], BF, name="cl", tag="cl")
                    nc.vector.tensor_scalar(cl[:], pb[:], -LAM, LAM,
                                            AluOpType.max, AluOpType.min)
                    ssb = bss.tile([128, 2, 2, 4, 2, 32], BF, name="ssb",
                                   tag="ssb")
                    nc.vector.tensor_tensor(ssb[:], pb[:], cl[:],
                                            AluOpType.subtract)
                    nc.vector.tensor_tensor(ytk[:, :, up], ssb[:, :, 0],
                                            ssb[:, :, 1], AluOpType.add)

                def do_l2_real(o1s, ytk):
                    """L2 + bias + softshrink for kw 0 (slot 0) and kw 128
                    (slot 1)."""
                    pb = bpb.tile([128, 2, 2, 4, 2, 32], BF, name="pb", tag="pb")
                    for kb in range(2):
                        sl = slice(kb * 128, (kb + 1) * 128)
                        p2 = p2p.tile([128, 2, 2 * C], FT, name="p2", tag="p2")
                        for u, o1u in enumerate(o1s):
                            nc.tensor.matmul(p2[:, u, 0:128], o1u[0][:, sl],
                                             w2_sb["w2_0"][:],
                                             start=True, stop=True)
                            nc.tensor.matmul(p2[:, u, 128:256], o1u[1][:, sl],
                                             w2_sb["w2_1"][:],
                                             start=True, stop=True)
                        nc.vector.tensor_tensor(pb[:, kb], p2[:],
                                                b2r_sb["b2rs"][:],
                                                AluOpType.add)
                    cl = bcl.tile([128, 2, 2, 4, 2, 32], BF, name="cl", tag="cl")
                    nc.vector.tensor_scalar(cl[:], pb[:], -LAM, LAM,
                                            AluOpType.max, AluOpType.min)
                    for u in range(2):
                        nc.vector.tensor_tensor(ytk[:, :, u], pb[:, :, u],
                                                cl[:, :, u], AluOpType.subtract)

                def do_ifft_pair(ytk, kwc0, nup=2):
                    """IFFT along H of nup combined columns; spill P, Q to
                    sp2.  ytk: (kh 128, kh-half 2, up 2, blk 4, comp 2, 32)."""
                    for hb in range(2):
                        hsl = slice(hb * 128, (hb + 1) * 128)
                        vps = psvp.tile([128, 2, 2, C], FT, name="vps", tag="vps")
                        first = True
                        for kh_h in range(2):
                            for ci, (m0, m1) in enumerate((("gr", "gn"),
                                                           ("gi", "gr"))):
                                nc.tensor.matmul(vps[:, ci, 0:nup, :],
                                                 g_sb[m0][kh_h][:, hsl],
                                                 ytk[:, kh_h, 0:nup, :, 0, :],
                                                 start=first, stop=False)
                                first = False
                                nc.tensor.matmul(vps[:, ci, 0:nup, :],
                                                 g_sb[m1][kh_h][:, hsl],
                                                 ytk[:, kh_h, 0:nup, :, 1, :],
                                                 start=False,
                                                 stop=(kh_h == 1 and ci == 1))
                        vsb = bv.tile([128, 2, 2, C], BF, name="vsb", tag="vsb")
                        nc.scalar.copy(
                            vsb[:, 0:nup, :, :],
                            vps[:, :, 0:nup, :].rearrange("p c u x -> p u c x"))
                        nc.gpsimd.dma_start(
                            sp2[hb * 128:(hb + 1) * 128, kwc0:kwc0 + nup, :, :],
                            vsb[:, 0:nup, :, :])

                def do_ifft_real(ytk):
                    """IFFT along H, real parts only, for kw 0 and kw 128."""
                    for hb in range(2):
                        hsl = slice(hb * 128, (hb + 1) * 128)
                        vps = psvp.tile([128, 2, 2, C], FT, name="vps", tag="vps")
                        for kh_h in range(2):
                            nc.tensor.matmul(vps[:, 0], g_sb["gr"][kh_h][:, hsl],
                                             ytk[:, kh_h, :, :, 0, :],
                                             start=(kh_h == 0), stop=False)
                            nc.tensor.matmul(vps[:, 0], g_sb["gn"][kh_h][:, hsl],
                                             ytk[:, kh_h, :, :, 1, :],
                                             start=False, stop=(kh_h == 1))
                        vsb = bv.tile([128, 2, 2, C], BF, name="vsb", tag="vsb")
                        nc.vector.tensor_copy(vsb[:, 0, 0, :], vps[:, 0, 0, :])
                        nc.scalar.copy(vsb[:, 1, 1, :], vps[:, 0, 1, :])
                        nc.gpsimd.dma_start(
                            sp2[hb * 128:(hb + 1) * 128, 0:1, 0:1, :],
                            vsb[:, 0:1, 0:1, :])
                        nc.gpsimd.dma_start(
                            sp2[hb * 128:(hb + 1) * 128, 128:129, 1:2, :],
                            vsb[:, 1:2, 1:2, :])

                for kg in range(17):
                    if kg < 15:
                        base, nj = 8 * kg + 1, 8
                    elif kg == 15:
                        base, nj = 121, 7
                    else:
                        base, nj = None, 2
                    bl = {}
                    if kg < 16:
                        for comp, src, row0 in (("r", sp1r, base),
                                                ("i", sp1i, base - 1)):
                            for hh in range(2):
                                t = bin_.tile([128, nj, C], BF,
                                              name=f"bl{comp}{hh}",
                                              tag=f"bl{comp}{hh}")
                                nc.sync.dma_start(
                                    t[:],
                                    src[row0:row0 + nj,
                                        hh * 128:(hh + 1) * 128, :]
                                    .rearrange("a b c -> b a c"),
                                )
                                bl[(comp, hh)] = t
                    else:
                        bsp = []
                        for hh in range(2):
                            t = bin_.tile([128, 2, C], BF, name=f"bsp{hh}",
                                          tag=f"blr{hh}")
                            nc.sync.dma_start(
                                t[:, 0:1, :],
                                sp1r[0:1, hh * 128:(hh + 1) * 128, :]
                                .rearrange("a b c -> b a c"))
                            nc.sync.dma_start(
                                t[:, 1:2, :],
                                sp1i[127:128, hh * 128:(hh + 1) * 128, :]
                                .rearrange("a b c -> b a c"))
                            bsp.append(t)

                    if kg < 16:
                        npair = nj // 2
                        for pj in range(npair):
                            ytk = byt.tile([128, 2, 2, 4, 2, 32], BF,
                                           name="ytk", tag="ytk")
                            for m in range(2):
                                jj = 2 * pj + m
                                s2 = do_fft_h(bl, jj)
                                o1 = do_l1(s2)
                                do_l2_unit(o1, ytk, m)
                            do_ifft_pair(ytk, base + 2 * pj)
                        if nj % 2 == 1:
                            jj = nj - 1
                            ytk = byt.tile([128, 2, 2, 4, 2, 32], BF,
                                           name="ytk", tag="ytk")
                            s2 = do_fft_h(bl, jj)
                            o1 = do_l1(s2)
                            do_l2_unit(o1, ytk, 0)
                            do_ifft_pair(ytk, base + jj, nup=1)
                        # prefetch this kg's finished columns into the
                        # resident kw-partition tiles (overlaps compute)
                        nc.sync.dma_start(
                            ctP[base:base + nj, :, :],
                            sp2[0:256, base:base + nj, 0, :]
                            .rearrange("h k c -> k h c"))
                        nc.sync.dma_start(
                            ctQ[base - 1:base - 1 + nj, :, :],
                            sp2[0:256, base:base + nj, 1, :]
                            .rearrange("h k c -> k h c"))
                    else:
                        ytk = byt.tile([128, 2, 2, 4, 2, 32], BF,
                                       name="ytk", tag="ytk")
                        s2a = do_fft_h_real(bsp, 0)
                        s2b = do_fft_h_real(bsp, 1)
                        o1s = do_l1_real(s2a, s2b)
                        do_l2_real(o1s, ytk)
                        do_ifft_real(ytk)
                        nc.sync.dma_start(
                            ctP[0:1, :, :],
                            sp2[0:256, 0:1, 0, :].rearrange("h k c -> k h c"))
                        nc.sync.dma_start(
                            ctQ[127:128, :, :],
                            sp2[0:256, 128:129, 1, :].rearrange("h k c -> k h c"))

            # ---------------- pass C: IFFT along W, real part ----------------
            with (
                tc.tile_pool(name="c_ev", bufs=8) as cev,
                tc.tile_pool(name="c_ps", bufs=6, space="PSUM") as cps,
            ):
                for g in range(32):          # 8 h-rows per group
                    for sub in range(2):
                        hsl = slice(8 * g + 4 * sub, 8 * g + 4 * sub + 4)
                        for wb in range(2):
                            ps = cps.tile([128, 4, C], FT, name="cps", tag="cps")
                            nc.tensor.matmul(
                                ps[:], cg_sb["cga"][:, wb * 128:(wb + 1) * 128],
                                ctP[:, hsl, :], start=True, stop=False)
                            nc.tensor.matmul(
                                ps[:], cg_sb["cgb"][:, wb * 128:(wb + 1) * 128],
                                ctQ[:, hsl, :], start=False, stop=True)
                            ev = cev.tile([128, 4, C], BF, name="cev", tag="cev")
                            if wb == 0:
                                nc.vector.tensor_copy(ev[:], ps[:])
                            else:
                                nc.scalar.copy(ev[:], ps[:])
                            weng = nc.gpsimd if wb == 0 else nc.sync
                            weng.dma_start(
                                out[wb * 128:(wb + 1) * 128,
                                    8 * g + 4 * sub:8 * g + 4 * sub + 4, :],
                                ev[:],
                            )

    nc.compile()
    return nc


_NC = None


def _get_nc():
    global _NC
    if _NC is None:
        _NC = _build()
    return _NC


def _host_constants():
    j = np.arange(256)
    ang = -2 * np.pi * np.outer(j, j) / 256
    Fr = (np.cos(ang) / 16).astype(np.float32)
    Fi = (np.sin(ang) / 16).astype(np.float32)
    Gr = Fr                     # cos even
    Gi = -Fi                    # inverse: conj
    bf = lambda a: np.ascontiguousarray(a).astype(ml_dtypes.bfloat16)
    return {
        "fwr": bf(Fr[:, 0:128]),
        "fwi2": bf(np.concatenate([Fi[:, 1:128], Fr[:, 128:129]], axis=1)),
        "fh1": bf(np.concatenate([Fr, Fi], axis=1)),
        "fh2": bf(np.concatenate([-Fi, Fr], axis=1)),
        "gr": bf(Gr), "gi": bf(Gi), "gn": bf(-Gi),
        "cga": bf(Fr[0:128, :]),
        "cgb": bf(np.concatenate([Fi[1:128, :], Fr[128:129, :]], axis=0)),
    }


def _host_weights(w1, b1, w2, b2, half):
    """Pack per-core (half) weights into the interleaved-block layout.

    L1 tile X (X = 0, 1) output partitions: [r_b | i_b | r_b' | i_b'] with
    (b, b') = (2X, 2X+1); a1 multiplies zr, a2 multiplies zi, a2m = mirror.
    L2 tile X: contract those partitions -> cols [re_b(32) | im_b(32) |
    re_b' | im_b'] (natural c order per comp after the IFFT comp-split).
    """
    blocks = slice(half * 4, (half + 1) * 4)
    w1r_b = w1[0, blocks]   # (4, 32, 32) [c_in, c_out]
    w1i_b = w1[1, blocks]
    w2r_b = w2[0, blocks]
    w2i_b = w2[1, blocks]
    b1r_b = b1[0, blocks]   # (4, 32)
    b1i_b = b1[1, blocks]
    b2r_b = b2[0, blocks]
    b2i_b = b2[1, blocks]

    bf = lambda a: np.ascontiguousarray(a).astype(ml_dtypes.bfloat16)

    def l1_mat(rmat, imat, X):
        # columns i of [C, C]: out partition i; rows = c_in (full 128, but
        # only the 32 rows of block b are nonzero)
        o = np.zeros((128, 128), np.float32)
        for s in range(2):          # sub-pair
            b = 2 * X + s
            ci = slice(b * 32, (b + 1) * 32)
            o[ci, s * 64:s * 64 + 32] = rmat[b]
            o[ci, s * 64 + 32:s * 64 + 64] = imat[b]
        return o

    def l2_mat(sign_im, X):
        # rows p: interleaved [r_b | i_b | r_b' | i_b']; cols j: [re_b |
        # im_b | re_b' | im_b'].  re = o1r.W2r - o1i.W2i;
        # im = sign_im * (o1i.W2r + o1r.W2i)
        o = np.zeros((128, 128), np.float32)
        for s in range(2):
            b = 2 * X + s
            r_rows = slice(s * 64, s * 64 + 32)
            i_rows = slice(s * 64 + 32, s * 64 + 64)
            re_cols = slice(s * 64, s * 64 + 32)
            im_cols = slice(s * 64 + 32, s * 64 + 64)
            o[r_rows, re_cols] = w2r_b[b]
            o[i_rows, re_cols] = -w2i_b[b]
            o[r_rows, im_cols] = sign_im * w2i_b[b]
            o[i_rows, im_cols] = sign_im * w2r_b[b]
        return o

    def b1_vec(X):
        v = np.empty(128, np.float32)
        for s in range(2):
            b = 2 * X + s
            v[s * 64:s * 64 + 32] = b1r_b[b]
            v[s * 64 + 32:s * 64 + 64] = b1i_b[b]
        return np.ascontiguousarray(v.reshape(128, 1))

    def b2_vec(sign_im):
        # p2 cols per slot: [re_b0 | im_b0 | re_b1 | im_b1 | re_b2 | ...]
        v = np.empty(256, np.float32)
        for b in range(4):
            v[b * 64:b * 64 + 32] = b2r_b[b]
            v[b * 64 + 32:b * 64 + 64] = sign_im * b2i_b[b]
        return v

    out = {
        "b1_0": b1_vec(0), "b1_1": b1_vec(1),
        "b2rn": bf(np.tile(np.concatenate([b2_vec(1), b2_vec(-1)])[None, :],
                           (128, 1))),
        "b2rs": bf(np.tile(np.concatenate([b2_vec(1), b2_vec(1)])[None, :],
                           (128, 1))),
    }
    for X in range(2):
        out[f"a1_{X}"] = bf(l1_mat(w1r_b, w1i_b, X))
        out[f"a2_{X}"] = bf(l1_mat(-w1i_b, w1r_b, X))
        out[f"a2m_{X}"] = bf(l1_mat(w1i_b, -w1r_b, X))
        out[f"w2_{X}"] = bf(l2_mat(1.0, X))
        out[f"w2m_{X}"] = bf(l2_mat(-1.0, X))
    return out


LAST_EXEC_TIME_NS = None
LAST_RESULT = None


def _prep_in_maps(x, w1, b1, w2, b2):
    x = np.asarray(x, dtype=np.float32)
    w1 = np.asarray(w1, dtype=np.float32)
    b1 = np.asarray(b1, dtype=np.float32)
    w2 = np.asarray(w2, dtype=np.float32)
    b2 = np.asarray(b2, dtype=np.float32)
    consts = _host_constants()
    in_maps = []
    for core in range(8):
        b, half = core // 2, core % 2
        xu = x[b].reshape(H, W, 256)[:, :, half * 128:(half + 1) * 128]
        xt = np.ascontiguousarray(xu.transpose(1, 0, 2)).astype(ml_dtypes.bfloat16)
        m = {"xt": xt}
        m.update(consts)
        m.update(_host_weights(w1, b1, w2, b2, half))
        in_maps.append(m)
    return in_maps


def _postprocess_out(o):
    # device emits (w, h, c); flatten to (h*w, c)
    return np.ascontiguousarray(
        o.reshape(W, H, C).transpose(1, 0, 2)).reshape(H * W, C)


def _expected_core_slice(expected, core):
    b, half = core // 2, core % 2
    return expected[b].reshape(65536, 256)[:, half * 128:(half + 1) * 128]


def kernel(x, w1, b1, w2, b2):
    nc = _get_nc()
    in_maps = _prep_in_maps(x, w1, b1, w2, b2)

    trace = os.environ.get("AFNO_TRACE", "0") == "1"
    tmpdir = os.environ.get("AFNO_TRACE_DIR") or None
    if tmpdir:
        os.makedirs(tmpdir, exist_ok=True)
    res = run_bass_kernel_spmd(nc, in_maps, list(range(8)), trace=trace,
                               tmpdir=tmpdir)
    global LAST_EXEC_TIME_NS, LAST_RESULT
    LAST_EXEC_TIME_NS = res.exec_time_ns
    LAST_RESULT = res

    full = np.empty((4, 65536, 256), dtype=np.float32)
    for core in range(8):
        b, half = core // 2, core % 2
        o = np.asarray(res.results[core]["out"]).astype(np.float32)
        full[b, :, half * 128:(half + 1) * 128] = _postprocess_out(o)
    return full


# revision 22
# speedup vs baseline: 1.0381x; 1.0381x over previous
"""AFNO2D (FFT2 -> block-diag complex MLP -> softshrink -> IFFT2) on 8 TRN2
NeuronCores.

Sharding: 32 independent (batch, channel-block) units; core i handles
(b = i // 2, channel half = i % 2) = 4 blocks of 32 channels.  No collectives.

Per-core Bass pipeline (all FFTs as TensorEngine matmuls against DFT
matrices, bf16 operands, fp32 PSUM accumulation).  Key structure: the final
real(IFFT2) only needs the Hermitian-symmetrized spectrum.  Since softshrink
is odd, the mirror column's conjugation folds into the MLP weights, and the
symmetrization U = S[kw] + conj(S[256-kw] @ kh-reversed) is a single add
after softshrink.

  pass A: FFT along W; Hermitian symmetry of the real-input FFT -> two
          128-row outputs: R = re kw 0..127, I = [im kw 1..127, re kw 128]
          (Nyquist folded into the main matmul) -> DRAM (kw, h, c)
  pass B: per kw 1..127: radix-2 FFT along H (even/odd h-halves, twiddle
          folded into the odd DFT matrix, butterfly on DVE doubles as the
          PSUM->SBUF move); MLP for the column and its mirror (mirror
          emitted at reversed-kh order so its input is conj(z), handled by
          sign-swapped weights).  L2 with o1 as lhsT lands (kh, c); bias is
          added by the DVE PSUM-drain (no bias matmul); softshrink
          clamp/sub on GpSimd; U = S0 + M1 on DVE.  IFFT along H batched
          over unit pairs -> CAST into h-partition staging -> SBUF->SBUF
          scatter DMA into resident (kw-partition) ctP/ctQ tiles.
  pass C: IFFT along W straight out of SBUF: out = cgA.P + cgB.Q with
          cgA = cos rows kw 0..127, cgB = [-sin rows kw 1..127; cos row
          128]; real out (h, w, c) -> DRAM.  No input DMA.
"""

import os
import sys

sys.path.insert(0, "/opt/trn_rl_repo")

import numpy as np
import ml_dtypes

import concourse.bacc as bacc
import concourse.mybir as mybir
from concourse import tile
from concourse.alu_op_type import AluOpType
from concourse.bass_utils import run_bass_kernel_spmd

FT = mybir.dt.float32
BF = mybir.dt.bfloat16
AF = mybir.ActivationFunctionType
H = 256
W = 256
C = 128
LAM = 0.01


def _build():
    nc = bacc.Bacc("TRN2", target_bir_lowering=False, debug=False, num_devices=8)

    xt = nc.declare_dram_parameter("xt", [W, H, C], BF, isOutput=False)
    fwr = nc.declare_dram_parameter("fwr", [W, 128], BF, isOutput=False)   # (w, kw)
    fwi2 = nc.declare_dram_parameter("fwi2", [W, 128], BF, isOutput=False)
    fh1 = nc.declare_dram_parameter("fh1", [H, 2 * H], BF, isOutput=False)  # [Fr|Fi]
    fh2 = nc.declare_dram_parameter("fh2", [H, 2 * H], BF, isOutput=False)  # [-Fi|Fr]
    gr = nc.declare_dram_parameter("gr", [H, H], BF, isOutput=False)
    gi = nc.declare_dram_parameter("gi", [H, H], BF, isOutput=False)
    gn = nc.declare_dram_parameter("gn", [H, H], BF, isOutput=False)       # -Gi
    cga = nc.declare_dram_parameter("cga", [128, W], BF, isOutput=False)
    cgb = nc.declare_dram_parameter("cgb", [128, W], BF, isOutput=False)
    # L1: mixed matrices producing (o1r, o1i) interleaved by 32-block on the
    # output partition dim: tile X covers blocks (2X, 2X+1) as
    # [r_b(32) | i_b(32) | r_b+1 | i_b+1].  a1 multiplies zr (shared by the
    # mirror column), a2 multiplies zi (a2m = mirror variant).
    wmats = {}
    for name in ["a1_0", "a1_1", "a2_0", "a2_1", "a2m_0", "a2m_1"]:
        wmats[name] = nc.declare_dram_parameter(name, [C, C], BF, isOutput=False)
    # L2: packed [interleaved o1 rows] -> [re_b(32) | im_b(32) | ...] cols
    w2d = {}
    for name in ["w2_0", "w2_1", "w2m_0", "w2m_1"]:
        w2d[name] = nc.declare_dram_parameter(name, [C, C], BF, isOutput=False)
    b1_d = {}
    for name in ["b1_0", "b1_1"]:
        b1_d[name] = nc.declare_dram_parameter(name, [C, 1], FT, isOutput=False)
    # b2 replicated across partitions: added by the DVE PSUM-drain
    b2r_d = {}
    for name in ["b2rn", "b2rs"]:
        b2r_d[name] = nc.declare_dram_parameter(name, [C, 4 * C], BF,
                                                isOutput=False)
    out = nc.declare_dram_parameter("out", [W, H, C], BF, isOutput=True)

    sp1r = nc.dram_tensor("sp1r", [128, H, C], BF)        # (kw 0..127, h, c)
    sp1i = nc.dram_tensor("sp1i", [128, H, C], BF)        # (im kw 1..127 | re 128)
    sp2 = nc.dram_tensor("sp2", [H, 129, 2, C], BF)       # (h, kwcol, {P,Q}, c)

    with tile.TileContext(nc) as tc:
        with tc.tile_pool(name="const", bufs=1) as cp:
            fw_sb = {"r": [], "i": []}
            for comp, src in (("r", fwr), ("i", fwi2)):
                for hh in range(2):
                    t = cp.tile([128, 128], BF, name=f"fw{comp}{hh}", tag=f"fw{comp}{hh}")
                    nc.sync.dma_start(t[:], src[hh * 128:(hh + 1) * 128, :])
                    fw_sb[comp].append(t)
            fh_sb = {1: [], 2: []}
            for idx, src in ((1, fh1), (2, fh2)):
                for hh in range(2):
                    t = cp.tile([128, 2 * H], BF, name=f"fh{idx}{hh}", tag=f"fh{idx}{hh}")
                    nc.gpsimd.dma_start(t[:], src[hh * 128:(hh + 1) * 128, :])
                    fh_sb[idx].append(t)
            g_sb = {}
            for nm, src in (("gr", gr), ("gi", gi), ("gn", gn)):
                g_sb[nm] = []
                for hh in range(2):
                    t = cp.tile([128, H], BF, name=f"{nm}{hh}", tag=f"{nm}{hh}")
                    nc.gpsimd.dma_start(t[:], src[hh * 128:(hh + 1) * 128, :])
                    g_sb[nm].append(t)
            cg_sb = {}
            for nm, src in (("cga", cga), ("cgb", cgb)):
                t = cp.tile([128, W], BF, name=nm, tag=nm)
                nc.gpsimd.dma_start(t[:], src[:])
                cg_sb[nm] = t
            w_sb = {}
            for nm, src in wmats.items():
                t = cp.tile([C, C], BF, name=nm, tag=nm)
                nc.gpsimd.dma_start(t[:], src[:])
                w_sb[nm] = t
            w2_sb = {}
            for nm, src in w2d.items():
                t = cp.tile([C, C], BF, name=nm, tag=nm)
                nc.gpsimd.dma_start(t[:], src[:])
                w2_sb[nm] = t
            b1_sb = {}
            for nm, src in b1_d.items():
                t = cp.tile([C, 1], FT, name=nm, tag=nm)
                nc.gpsimd.dma_start(t[:], src[:])
                b1_sb[nm] = t
            b2r_sb = {}
            for nm, src in b2r_d.items():
                t = cp.tile([C, 4 * C], BF, name=nm, tag=nm)
                nc.gpsimd.dma_start(t[:], src[:])
                b2r_sb[nm] = t
            # resident IFFT-H outputs, pass-C layout (kw-partition):
            #   ctP rows = kw 0..127 (P comp), ctQ rows = kw 1..128 (Q comp)
            ctP = cp.tile([128, H, C], BF, name="ctP", tag="ctP")
            ctQ = cp.tile([128, H, C], BF, name="ctQ", tag="ctQ")

            # ---------------- pass A: FFT along W ----------------
            with (
                tc.tile_pool(name="a_in", bufs=4) as ain,
                tc.tile_pool(name="a_ev", bufs=8) as aev,
                tc.tile_pool(name="a_ps", bufs=6, space="PSUM") as aps,
            ):
                for g in range(32):           # 8 h-rows per group
                    xr = []
                    for wh in range(2):
                        t = ain.tile([128, 8, C], BF, name=f"xr{wh}", tag=f"xr{wh}")
                        eng = nc.sync if wh == 0 else nc.scalar
                        eng.dma_start(
                            t[:], xt[wh * 128:(wh + 1) * 128, 8 * g:8 * g + 8, :]
                        )
                        xr.append(t)
                    evs = {}
                    for comp in ("r", "i"):
                        evs[comp] = aev.tile([128, 8, C], BF, name=f"aev{comp}",
                                             tag=f"aev{comp}")
                    k = 0
                    for comp in ("r", "i"):
                        for sub in range(2):
                            ps = aps.tile([128, 4, C], FT, name="aps", tag="aps")
                            for wh in range(2):
                                nc.tensor.matmul(
                                    ps[:],
                                    fw_sb[comp][wh][:],
                                    xr[wh][:, 4 * sub:4 * sub + 4, :],
                                    start=(wh == 0),
                                    stop=(wh == 1),
                                )
                            if k % 2 == 0:
                                nc.vector.tensor_copy(
                                    evs[comp][:, 4 * sub:4 * sub + 4, :], ps[:])
                            else:
                                nc.scalar.copy(
                                    evs[comp][:, 4 * sub:4 * sub + 4, :], ps[:])
                            k += 1
                    weng = nc.sync if g >= 28 else nc.gpsimd
                    for comp, dst in (("r", sp1r), ("i", sp1i)):
                        weng.dma_start(
                            dst[0:128, 8 * g:8 * g + 8, :], evs[comp][:])

            # ---------------- pass B ----------------
            with (
                tc.tile_pool(name="b_in", bufs=2) as bin_,
                tc.tile_pool(name="b_s2", bufs=2) as bs2,
                tc.tile_pool(name="b_o1", bufs=3) as bo1,
                tc.tile_pool(name="b_pb", bufs=3) as bpb,
                tc.tile_pool(name="b_cl", bufs=3) as bcl,
                tc.tile_pool(name="b_ss", bufs=3) as bss,
                tc.tile_pool(name="b_yt", bufs=2) as byt,
                tc.tile_pool(name="b_v", bufs=4) as bv,
                tc.tile_pool(name="b_ps2", bufs=2, space="PSUM") as ps2p,
                tc.tile_pool(name="b_p1", bufs=1, space="PSUM") as p1p,
                tc.tile_pool(name="b_p2", bufs=2, space="PSUM") as p2p,
                tc.tile_pool(name="b_psv", bufs=2, space="PSUM") as psvp,
            ):

                def do_fft_h(bl, jj):
                    """FFT along H for one kw column -> s2 (c, [re 256|im 256])."""
                    ps = ps2p.tile([128, 2 * H], FT, name="s2ps", tag="s2ps")
                    nc.tensor.matmul(ps[:], bl[("r", 0)][:, jj, :],
                                     fh_sb[1][0][:], start=True, stop=False)
                    nc.tensor.matmul(ps[:], bl[("i", 0)][:, jj, :],
                                     fh_sb[2][0][:], start=False, stop=False)
                    nc.tensor.matmul(ps[:], bl[("r", 1)][:, jj, :],
                                     fh_sb[1][1][:], start=False, stop=False)
                    nc.tensor.matmul(ps[:], bl[("i", 1)][:, jj, :],
                                     fh_sb[2][1][:], start=False, stop=True)
                    s2 = bs2.tile([128, 2, H], BF, name="s2", tag="s2")
                    nc.scalar.copy(s2[:], ps[:])
                    return s2

                def do_fft_h_real(bsp, sl):
                    """FFT along H of a real column (bsp[:, sl])."""
                    ps = ps2p.tile([128, 2 * H], FT, name="s2ps", tag="s2ps")
                    nc.tensor.matmul(ps[:], bsp[0][:, sl, :], fh_sb[1][0][:],
                                     start=True, stop=False)
                    nc.tensor.matmul(ps[:], bsp[1][:, sl, :], fh_sb[1][1][:],
                                     start=False, stop=True)
                    s2 = bs2.tile([128, 2, H], BF, name="s2", tag="s2")
                    nc.scalar.copy(s2[:], ps[:])
                    return s2

                def do_l1(s2):
                    """L1 for a column and its mirror.  p1a = (normal-01,
                    mirror-01) sharing bias b1_0 -> one batched relu; p1b =
                    (normal-23, mirror-23) with b1_1."""
                    zr, zi = s2[:, 0, :], s2[:, 1, :]
                    p1a = p1p.tile([128, 2, H], FT, name="p1a", tag="p1a")
                    nc.tensor.matmul(p1a[:, 0], w_sb["a1_0"][:], zr,
                                     start=True, stop=False)
                    nc.tensor.matmul(p1a[:, 1], w_sb["a1_0"][:], zr,
                                     start=False, stop=False)
                    nc.tensor.matmul(p1a[:, 0], w_sb["a2_0"][:], zi,
                                     start=False, stop=False)
                    nc.tensor.matmul(p1a[:, 1], w_sb["a2m_0"][:], zi,
                                     start=False, stop=True)
                    p1b = p1p.tile([128, 2, H], FT, name="p1b", tag="p1b")
                    nc.tensor.matmul(p1b[:, 0], w_sb["a1_1"][:], zr,
                                     start=True, stop=False)
                    nc.tensor.matmul(p1b[:, 1], w_sb["a1_1"][:], zr,
                                     start=False, stop=False)
                    nc.tensor.matmul(p1b[:, 0], w_sb["a2_1"][:], zi,
                                     start=False, stop=False)
                    nc.tensor.matmul(p1b[:, 1], w_sb["a2m_1"][:], zi,
                                     start=False, stop=True)
                    o1a = bo1.tile([128, 2, H], BF, name="o1a", tag="o1a")
                    nc.scalar.activation(o1a[:], p1a[:], AF.Relu,
                                         bias=b1_sb["b1_0"][:])
                    o1b = bo1.tile([128, 2, H], BF, name="o1b", tag="o1b")
                    nc.scalar.activation(o1b[:], p1b[:], AF.Relu,
                                         bias=b1_sb["b1_1"][:])
                    return (o1a, o1b)

                def do_l1_real(s2a, s2b):
                    """L1 for the two real-input columns (kw 0 and 128): no
                    mirror, but z = FFT_H(re) is still complex."""
                    o1 = []
                    for s2 in (s2a, s2b):
                        zr, zi = s2[:, 0, :], s2[:, 1, :]
                        p1a = p1p.tile([128, 2, H], FT, name="p1a", tag="p1a")
                        nc.tensor.matmul(p1a[:, 0], w_sb["a1_0"][:], zr,
                                         start=True, stop=False)
                        nc.tensor.matmul(p1a[:, 1], w_sb["a1_1"][:], zr,
                                         start=False, stop=False)
                        nc.tensor.matmul(p1a[:, 0], w_sb["a2_0"][:], zi,
                                         start=False, stop=False)
                        nc.tensor.matmul(p1a[:, 1], w_sb["a2_1"][:], zi,
                                         start=False, stop=True)
                        o1u = []
                        for x in range(2):
                            t = bo1.tile([128, H], BF, name=f"o1n{x}",
                                         tag=f"o1n{x}")
                            nc.scalar.activation(t[:], p1a[:, x], AF.Relu,
                                                 bias=b1_sb[f"b1_{x}"][:])
                            o1u.append(t)
                        o1.append(o1u)
                    return o1

                def do_l2_unit(o1, ytk, up):
                    """L2 + bias + softshrink + U for one (column, mirror)
                    unit.  p2 cols per slot: (b: re32 | im32) x 4 blocks.
                    Bias is applied by the DVE PSUM-drain (no bias matmul);
                    clamp/sub run on GpSimd over both kh-halves at once."""
                    o1a, o1b = o1
                    pb = bpb.tile([128, 2, 2, 4, 2, 32], BF, name="pb", tag="pb")
                    for kb in range(2):
                        sl = slice(kb * 128, (kb + 1) * 128)
                        p2 = p2p.tile([128, 2, 2 * C], FT, name="p2", tag="p2")
                        nc.tensor.matmul(p2[:, 0, 0:128], o1a[:, 0, sl],
                                         w2_sb["w2_0"][:], start=True, stop=True)
                        nc.tensor.matmul(p2[:, 0, 128:256], o1b[:, 0, sl],
                                         w2_sb["w2_1"][:], start=True, stop=True)
                        nc.tensor.matmul(p2[:, 1, 0:128], o1a[:, 1, sl],
                                         w2_sb["w2m_0"][:], start=True, stop=True)
                        nc.tensor.matmul(p2[:, 1, 128:256], o1b[:, 1, sl],
                                         w2_sb["w2m_1"][:], start=True, stop=True)
                        nc.vector.tensor_tensor(pb[:, kb], p2[:],
                                                b2r_sb["b2rn"][:],
                                                AluOpType.add)
                    cl = bcl.tile([128, 2, 2, 4, 2, 32], BF, name="cl", tag="cl")
                    nc.vector.tensor_scalar(cl[:], pb[:], -LAM, LAM,
                                            AluOpType.max, AluOpType.min)
                    ssb = bss.tile([128, 2, 2, 4, 2, 32], BF, name="ssb",
                                   tag="ssb")
                    nc.vector.tensor_tensor(ssb[:], pb[:], cl[:],
                                            AluOpType.subtract)
                    nc.vector.tensor_tensor(ytk[:, :, up], ssb[:, :, 0],
                                            ssb[:, :, 1], AluOpType.add)

                def do_l2_real(o1s, ytk):
                    """L2 + bias + softshrink for kw 0 (slot 0) and kw 128
                    (slot 1)."""
                    pb = bpb.tile([128, 2, 2, 4, 2, 32], BF, name="pb", tag="pb")
                    for kb in range(2):
                        sl = slice(kb * 128, (kb + 1) * 128)
                        p2 = p2p.tile([128, 2, 2 * C], FT, name="p2", tag="p2")
                        for u, o1u in enumerate(o1s):
                            nc.tensor.matmul(p2[:, u, 0:128], o1u[0][:, sl],
                                             w2_sb["w2_0"][:],
                                             start=True, stop=True)
                            nc.tensor.matmul(p2[:, u, 128:256], o1u[1][:, sl],
                                             w2_sb["w2_1"][:],
                                             start=True, stop=True)
                        nc.vector.tensor_tensor(pb[:, kb], p2[:],
                                                b2r_sb["b2rs"][:],
                                                AluOpType.add)
                    cl = bcl.tile([128, 2, 2, 4, 2, 32], BF, name="cl", tag="cl")
                    nc.vector.tensor_scalar(cl[:], pb[:], -LAM, LAM,
                                            AluOpType.max, AluOpType.min)
                    for u in range(2):
                        nc.vector.tensor_tensor(ytk[:, :, u], pb[:, :, u],
                                                cl[:, :, u], AluOpType.subtract)

                def do_ifft_pair(ytk, kwc0, nup=2):
                    """IFFT along H of nup combined columns; spill P, Q to
                    sp2.  ytk: (kh 128, kh-half 2, up 2, blk 4, comp 2, 32)."""
                    for hb in range(2):
                        hsl = slice(hb * 128, (hb + 1) * 128)
                        vps = psvp.tile([128, 2, 2, C], FT, name="vps", tag="vps")
                        first = True
                        for kh_h in range(2):
                            for ci, (m0, m1) in enumerate((("gr", "gn"),
                                                           ("gi", "gr"))):
                                nc.tensor.matmul(vps[:, ci, 0:nup, :],
                                                 g_sb[m0][kh_h][:, hsl],
                                                 ytk[:, kh_h, 0:nup, :, 0, :],
                                                 start=first, stop=False)
                                first = False
                                nc.tensor.matmul(vps[:, ci, 0:nup, :],
                                                 g_sb[m1][kh_h][:, hsl],
                                                 ytk[:, kh_h, 0:nup, :, 1, :],
                                                 start=False,
                                                 stop=(kh_h == 1 and ci == 1))
                        vsb = bv.tile([128, 2, 2, C], BF, name="vsb", tag="vsb")
                        nc.scalar.copy(
                            vsb[:, 0:nup, :, :],
                            vps[:, :, 0:nup, :].rearrange("p c u x -> p u c x"))
                        nc.gpsimd.dma_start(
                            sp2[hb * 128:(hb + 1) * 128, kwc0:kwc0 + nup, :, :],
                            vsb[:, 0:nup, :, :])

                def do_ifft_real(ytk):
                    """IFFT along H, real parts only, for kw 0 and kw 128."""
                    for hb in range(2):
                        hsl = slice(hb * 128, (hb + 1) * 128)
                        vps = psvp.tile([128, 2, 2, C], FT, name="vps", tag="vps")
                        for kh_h in range(2):
                            nc.tensor.matmul(vps[:, 0], g_sb["gr"][kh_h][:, hsl],
                                             ytk[:, kh_h, :, :, 0, :],
                                             start=(kh_h == 0), stop=False)
                            nc.tensor.matmul(vps[:, 0], g_sb["gn"][kh_h][:, hsl],
                                             ytk[:, kh_h, :, :, 1, :],
                                             start=False, stop=(kh_h == 1))
                        vsb = bv.tile([128, 2, 2, C], BF, name="vsb", tag="vsb")
                        nc.vector.tensor_copy(vsb[:, 0, 0, :], vps[:, 0, 0, :])
                        nc.scalar.copy(vsb[:, 1, 1, :], vps[:, 0, 1, :])
                        nc.gpsimd.dma_start(
                            sp2[hb * 128:(hb + 1) * 128, 0:1, 0:1, :],
                            vsb[:, 0:1, 0:1, :])
                        nc.gpsimd.dma_start(
                            sp2[hb * 128:(hb + 1) * 128, 128:129, 1:2, :],
                            vsb[:, 1:2, 1:2, :])

                for kg in range(17):
                    if kg < 15:
                        base, nj = 8 * kg + 1, 8
                    elif kg == 15:
                        base, nj = 121, 7
                    else:
                        base, nj = None, 2
                    bl = {}
                    if kg < 16:
                        for comp, src, row0 in (("r", sp1r, base),
                                                ("i", sp1i, base - 1)):
                            for hh in range(2):
                                t = bin_.tile([128, nj, C], BF,
                                              name=f"bl{comp}{hh}",
                                              tag=f"bl{comp}{hh}")
                                nc.sync.dma_start(
                                    t[:],
                                    src[row0:row0 + nj,
                                        hh * 128:(hh + 1) * 128, :]
                                    .rearrange("a b c -> b a c"),
                                )
                                bl[(comp, hh)] = t
                    else:
                        bsp = []
                        for hh in range(2):
                            t = bin_.tile([128, 2, C], BF, name=f"bsp{hh}",
                                          tag=f"blr{hh}")
                            nc.sync.dma_start(
                                t[:, 0:1, :],
                                sp1r[0:1, hh * 128:(hh + 1) * 128, :]
                                .rearrange("a b c -> b a c"))
                            nc.sync.dma_start(
                                t[:, 1:2, :],
                                sp1i[127:128, hh * 128:(hh + 1) * 128, :]
                                .rearrange("a b c -> b a c"))
                            bsp.append(t)

                    if kg < 16:
                        npair = nj // 2
                        for pj in range(npair):
                            ytk = byt.tile([128, 2, 2, 4, 2, 32], BF,
                                           name="ytk", tag="ytk")
                            for m in range(2):
                                jj = 2 * pj + m
                                s2 = do_fft_h(bl, jj)
                                o1 = do_l1(s2)
                                do_l2_unit(o1, ytk, m)
                            do_ifft_pair(ytk, base + 2 * pj)
                        if nj % 2 == 1:
                            jj = nj - 1
                            ytk = byt.tile([128, 2, 2, 4, 2, 32], BF,
                                           name="ytk", tag="ytk")
                            s2 = do_fft_h(bl, jj)
                            o1 = do_l1(s2)
                            do_l2_unit(o1, ytk, 0)
                            do_ifft_pair(ytk, base + jj, nup=1)
                        # prefetch this kg's finished columns into the
                        # resident kw-partition tiles (overlaps compute)
                        nc.gpsimd.dma_start(
                            ctP[base:base + nj, :, :],
                            sp2[0:256, base:base + nj, 0, :]
                            .rearrange("h k c -> k h c"))
                        nc.gpsimd.dma_start(
                            ctQ[base - 1:base - 1 + nj, :, :],
                            sp2[0:256, base:base + nj, 1, :]
                            .rearrange("h k c -> k h c"))
                    else:
                        ytk = byt.tile([128, 2, 2, 4, 2, 32], BF,
                                       name="ytk", tag="ytk")
                        s2a = do_fft_h_real(bsp, 0)
                        s2b = do_fft_h_real(bsp, 1)
                        o1s = do_l1_real(s2a, s2b)
                        do_l2_real(o1s, ytk)
                        do_ifft_real(ytk)
                        nc.gpsimd.dma_start(
                            ctP[0:1, :, :],
                            sp2[0:256, 0:1, 0, :].rearrange("h k c -> k h c"))
                        nc.gpsimd.dma_start(
                            ctQ[127:128, :, :],
                            sp2[0:256, 128:129, 1, :].rearrange("h k c -> k h c"))

            # ---------------- pass C: IFFT along W, real part ----------------
            with (
                tc.tile_pool(name="c_ev", bufs=8) as cev,
                tc.tile_pool(name="c_ps", bufs=6, space="PSUM") as cps,
            ):
                for g in range(32):          # 8 h-rows per group
                    for sub in range(2):
                        hsl = slice(8 * g + 4 * sub, 8 * g + 4 * sub + 4)
                        for wb in range(2):
                            ps = cps.tile([128, 4, C], FT, name="cps", tag="cps")
                            nc.tensor.matmul(
                                ps[:], cg_sb["cga"][:, wb * 128:(wb + 1) * 128],
                                ctP[:, hsl, :], start=True, stop=False)
                            nc.tensor.matmul(
                                ps[:], cg_sb["cgb"][:, wb * 128:(wb + 1) * 128],
                                ctQ[:, hsl, :], start=False, stop=True)
                            ev = cev.tile([128, 4, C], BF, name="cev", tag="cev")
                            if wb == 0:
                                nc.vector.tensor_copy(ev[:], ps[:])
                            else:
                                nc.scalar.copy(ev[:], ps[:])
                            weng = nc.gpsimd if wb == 0 else nc.sync
                            weng.dma_start(
                                out[wb * 128:(wb + 1) * 128,
                                    8 * g + 4 * sub:8 * g + 4 * sub + 4, :],
                                ev[:],
                            )

    nc.compile()
    return nc


_NC = None


def _get_nc():
    global _NC
    if _NC is None:
        _NC = _build()
    return _NC


def _host_constants():
    j = np.arange(256)
    ang = -2 * np.pi * np.outer(j, j) / 256
    Fr = (np.cos(ang) / 16).astype(np.float32)
    Fi = (np.sin(ang) / 16).astype(np.float32)
    Gr = Fr                     # cos even
    Gi = -Fi                    # inverse: conj
    bf = lambda a: np.ascontiguousarray(a).astype(ml_dtypes.bfloat16)
    return {
        "fwr": bf(Fr[:, 0:128]),
        "fwi2": bf(np.concatenate([Fi[:, 1:128], Fr[:, 128:129]], axis=1)),
        "fh1": bf(np.concatenate([Fr, Fi], axis=1)),
        "fh2": bf(np.concatenate([-Fi, Fr], axis=1)),
        "gr": bf(Gr), "gi": bf(Gi), "gn": bf(-Gi),
        "cga": bf(Fr[0:128, :]),
        "cgb": bf(np.concatenate([Fi[1:128, :], Fr[128:129, :]], axis=0)),
    }


def _host_weights(w1, b1, w2, b2, half):
    """Pack per-core (half) weights into the interleaved-block layout.

    L1 tile X (X = 0, 1) output partitions: [r_b | i_b | r_b' | i_b'] with
    (b, b') = (2X, 2X+1); a1 multiplies zr, a2 multiplies zi, a2m = mirror.
    L2 tile X: contract those partitions -> cols [re_b(32) | im_b(32) |
    re_b' | im_b'] (natural c order per comp after the IFFT comp-split).
    """
    blocks = slice(half * 4, (half + 1) * 4)
    w1r_b = w1[0, blocks]   # (4, 32, 32) [c_in, c_out]
    w1i_b = w1[1, blocks]
    w2r_b = w2[0, blocks]
    w2i_b = w2[1, blocks]
    b1r_b = b1[0, blocks]   # (4, 32)
    b1i_b = b1[1, blocks]
    b2r_b = b2[0, blocks]
    b2i_b = b2[1, blocks]

    bf = lambda a: np.ascontiguousarray(a).astype(ml_dtypes.bfloat16)

    def l1_mat(rmat, imat, X):
        # columns i of [C, C]: out partition i; rows = c_in (full 128, but
        # only the 32 rows of block b are nonzero)
        o = np.zeros((128, 128), np.float32)
        for s in range(2):          # sub-pair
            b = 2 * X + s
            ci = slice(b * 32, (b + 1) * 32)
            o[ci, s * 64:s * 64 + 32] = rmat[b]
            o[ci, s * 64 + 32:s * 64 + 64] = imat[b]
        return o

    def l2_mat(sign_im, X):
        # rows p: interleaved [r_b | i_b | r_b' | i_b']; cols j: [re_b |
        # im_b | re_b' | im_b'].  re = o1r.W2r - o1i.W2i;
        # im = sign_im * (o1i.W2r + o1r.W2i)
        o = np.zeros((128, 128), np.float32)
        for s in range(2):
            b = 2 * X + s
            r_rows = slice(s * 64, s * 64 + 32)
            i_rows = slice(s * 64 + 32, s * 64 + 64)
            re_cols = slice(s * 64, s * 64 + 32)
            im_cols = slice(s * 64 + 32, s * 64 + 64)
            o[r_rows, re_cols] = w2r_b[b]
            o[i_rows, re_cols] = -w2i_b[b]
            o[r_rows, im_cols] = sign_im * w2i_b[b]
            o[i_rows, im_cols] = sign_im * w2r_b[b]
        return o

    def b1_vec(X):
        v = np.empty(128, np.float32)
        for s in range(2):
            b = 2 * X + s
            v[s * 64:s * 64 + 32] = b1r_b[b]
            v[s * 64 + 32:s * 64 + 64] = b1i_b[b]
        return np.ascontiguousarray(v.reshape(128, 1))

    def b2_vec(sign_im):
        # p2 cols per slot: [re_b0 | im_b0 | re_b1 | im_b1 | re_b2 | ...]
        v = np.empty(256, np.float32)
        for b in range(4):
            v[b * 64:b * 64 + 32] = b2r_b[b]
            v[b * 64 + 32:b * 64 + 64] = sign_im * b2i_b[b]
        return v

    out = {
        "b1_0": b1_vec(0), "b1_1": b1_vec(1),
        "b2rn": bf(np.tile(np.concatenate([b2_vec(1), b2_vec(-1)])[None, :],
                           (128, 1))),
        "b2rs": bf(np.tile(np.concatenate([b2_vec(1), b2_vec(1)])[None, :],
                           (128, 1))),
    }
    for X in range(2):
        out[f"a1_{X}"] = bf(l1_mat(w1r_b, w1i_b, X))
        out[f"a2_{X}"] = bf(l1_mat(-w1i_b, w1r_b, X))
        out[f"a2m_{X}"] = bf(l1_mat(w1i_b, -w1r_b, X))
        out[f"w2_{X}"] = bf(l2_mat(1.0, X))
        out[f"w2m_{X}"] = bf(l2_mat(-1.0, X))
    return out


LAST_EXEC_TIME_NS = None
LAST_RESULT = None


def _prep_in_maps(x, w1, b1, w2, b2):
    x = np.asarray(x, dtype=np.float32)
    w1 = np.asarray(w1, dtype=np.float32)
    b1 = np.asarray(b1, dtype=np.float32)
    w2 = np.asarray(w2, dtype=np.float32)
    b2 = np.asarray(b2, dtype=np.float32)
    consts = _host_constants()
    in_maps = []
    for core in range(8):
        b, half = core // 2, core % 2
        xu = x[b].reshape(H, W, 256)[:, :, half * 128:(half + 1) * 128]
        xt = np.ascontiguousarray(xu.transpose(1, 0, 2)).astype(ml_dtypes.bfloat16)
        m = {"xt": xt}
        m.update(consts)
        m.update(_host_weights(w1, b1, w2, b2, half))
        in_maps.append(m)
    return in_maps


def _postprocess_out(o):
    # device emits (w, h, c); flatten to (h*w, c)
    return np.ascontiguousarray(
        o.reshape(W, H, C).transpose(1, 0, 2)).reshape(H * W, C)


def _expected_core_slice(expected, core):
    b, half = core // 2, core % 2
    return expected[b].reshape(65536, 256)[:, half * 128:(half + 1) * 128]


def kernel(x, w1, b1, w2, b2):
    nc = _get_nc()
    in_maps = _prep_in_maps(x, w1, b1, w2, b2)

    trace = os.environ.get("AFNO_TRACE", "0") == "1"
    tmpdir = os.environ.get("AFNO_TRACE_DIR") or None
    if tmpdir:
        os.makedirs(tmpdir, exist_ok=True)
    res = run_bass_kernel_spmd(nc, in_maps, list(range(8)), trace=trace,
                               tmpdir=tmpdir)
    global LAST_EXEC_TIME_NS, LAST_RESULT
    LAST_EXEC_TIME_NS = res.exec_time_ns
    LAST_RESULT = res

    full = np.empty((4, 65536, 256), dtype=np.float32)
    for core in range(8):
        b, half = core // 2, core % 2
        o = np.asarray(res.results[core]["out"]).astype(np.float32)
        full[b, :, half * 128:(half + 1) * 128] = _postprocess_out(o)
    return full
